# revision 58
# baseline (speedup 1.0000x reference)
"""Trainium2 Bass kernel for BaselineFeedforwardNetwork recurrence.

Reference computation (per path, T=60 steps, serial in t):
    x_t = [features_t (8), delta_{t-1} (1)]            # (9,)
    h1  = relu(x_t @ W1 + b1)                          # (128,)
    h2  = relu(h1 @ W2 + b2)                           # (128,)
    d_t = h2 @ W3 + b3                                 # (1,)
Output: deltas (N, T).

Data-parallel over N=65536 paths across 8 NeuronCores (8192/core),
weights replicated, recurrence local per core.

V2 per-core dataflow (bf16 matmuls, fp32 PSUM), 16 path-tiles of 512
(tile (s,g): paths 2048s+512g+c), per step, 48 matmuls (vs v1's 64):
- mm1: ONE K=9 matmul per tile (merged features+delta): lhsT = W1p rows
  32g..32g+8 (8 feature rows + delta-weight row), rhs = feature chunk
  tile rows 32g..32g+8. The delta row 32g+8 of the feature tile is
  populated per step by contiguous [4,512]->[1,2048] insert-DMAs from
  ds; chunk 0's delta rows are zero-filled from DRAM (t=0: delta0=0).
- mm2: full K=128 matmul per tile.
- mm3: K=128, lhsT = shifted single-column W3 slice -> psum row
  112+4g+s; groups 0-1 accumulate in pd bank A, groups 2-3 in bank B.
- Each pd half is evicted (+b3, on DVE) as soon as its 8 mm3s finish,
  then immediately fires its 2 insert-DMAs (sync queue) + out-DMA, so
  the t->t+1 delta chain overlaps the other half's compute.
- Software pipeline: PE issue order mm1(tau) / mm2(tau-3) / mm3(tau-7)
  with a tile ring, crossing step boundaries, so eviction waits never
  head-of-line-block ready mm1s in the in-order PE queue.
- Evictions (PSUM->SBUF + bias + relu): h1/h2 split ACT 17 / DVE 15
  per step by tile parity; both pd evicts on DVE (least convoying).
PSUM: ph1 3 banks + ph2 3 + pd 2 = 8. Chunk feature loads (10 steps —
shorter Pool-queue DMA slices interleave better than 15) on the gpsimd
queue; ds-sourced DMAs on sync (gpsimd DMA reads lack cross-engine deps
in Tile).
"""

import os
import sys

import numpy as np

for _p in ("/opt/trn_rl_repo", "/root/.axon_site/_ro/trn_rl_repo"):
    if _p not in sys.path and os.path.isdir(_p):
        sys.path.append(_p)

import ml_dtypes  # noqa: E402

N_FULL = 65536
T_FULL = 60
F = 8
HID = 128
N_CORES = 8
NSH = N_FULL // N_CORES  # 8192 paths per core

BF16 = ml_dtypes.bfloat16

VARIANT = "v2"


def build_kernel_v2(nsh=NSH, t_steps=T_FULL, num_cores=N_CORES, b3_value=0.0,
                    chunk_steps=10, repeats=1, ph1_bufs=3, ph2_bufs=3,
                    h1_bufs=7, h2_bufs=5, lag1=3, lag2=7,
                    alt_balance=False, pd_swap='dve', late_out=False,
                    dst_bufs=2, flip=False, extra_act=(15,),
                    lag2a=None, act_h2=(), split_h2=(), pair_ph2=False,
                    lag2t=None, pd_bufs=2, pd_split=False):
    """V2 per-core Bass graph. Returns the compiled nc."""
    import concourse.bass as bass
    import concourse.tile as tile
    from concourse import bacc, mybir

    bf = mybir.dt.bfloat16
    f32 = mybir.dt.float32
    NT = 512
    ntiles = nsh // NT             # 16
    npacks = ntiles // 4           # 4
    xw = npacks * NT               # 2048
    TC = min(chunk_steps, t_steps)
    assert ntiles == 16

    nc = bacc.Bacc("TRN2", target_bir_lowering=False, debug=False,
                   num_devices=num_cores)

    feat = nc.declare_dram_parameter("features", [4, F + 1, t_steps, xw], bf,
                                     isOutput=False)
    w1p = nc.declare_dram_parameter("W1p", [128, HID], bf, isOutput=False)
    w2 = nc.declare_dram_parameter("W2", [HID, HID], bf, isOutput=False)
    w3b = nc.declare_dram_parameter("W3b", [128, 144], bf, isOutput=False)
    b1 = nc.declare_dram_parameter("b1", [128, 1], f32, isOutput=False)
    b2 = nc.declare_dram_parameter("b2", [128, 1], f32, isOutput=False)
    out = nc.declare_dram_parameter("out", [t_steps, 16, NT], bf,
                                    isOutput=True)

    Relu = mybir.ActivationFunctionType.Relu
    Copy = mybir.ActivationFunctionType.Copy
    add = mybir.AluOpType.add
    amax = mybir.AluOpType.max

    with tile.TileContext(nc) as tc:
        with (
            tc.tile_pool(name="consts", bufs=1) as cpool,
            tc.tile_pool(name="f", bufs=2) as fpool,
            tc.tile_pool(name="h1r", bufs=h1_bufs) as h1pool,
            tc.tile_pool(name="h2r", bufs=h2_bufs) as h2pool,
            tc.tile_pool(name="dst", bufs=dst_bufs) as dpool,
            tc.tile_pool(name="ph1", bufs=ph1_bufs, space="PSUM") as ph1pool,
            tc.tile_pool(name="ph2", bufs=ph2_bufs, space="PSUM") as ph2pool,
            tc.tile_pool(name="pd", bufs=pd_bufs, space="PSUM") as pdpool,
        ):
            w1sb = cpool.tile([128, HID], bf, tag="w1")
            w2sb = cpool.tile([HID, HID], bf, tag="w2")
            w3sb = cpool.tile([128, 144], bf, tag="w3")
            b1sb = cpool.tile([128, 1], f32, tag="b1")
            b2sb = cpool.tile([128, 1], f32, tag="b2")
            nc.gpsimd.dma_start(w1sb[:], w1p[:])
            nc.gpsimd.dma_start(w2sb[:], w2[:])
            nc.gpsimd.dma_start(w3sb[:], w3b[:])
            nc.gpsimd.dma_start(b1sb[:], b1[:])
            nc.gpsimd.dma_start(b2sb[:], b2[:])

            def load_chunk(ci):
                t0 = ci * TC
                tl = min(TC, t_steps - t0)
                ftile = fpool.tile([128, TC * xw], bf, tag="f")
                for g in range(4):
                    nc.gpsimd.dma_start(
                        ftile[32 * g:32 * g + 8, 0:tl * xw],
                        feat[g, 0:F, t0:t0 + tl, 0:xw],
                    )
                    if ci == 0:
                        # chunk 0's delta rows must be zeros for t=0 (the
                        # K=9 mm1 reads them); later chunks' delta rows are
                        # first written by the per-step insert-DMAs.
                        nc.sync.dma_start(
                            ftile[32 * g + 8:32 * g + 9, 0:tl * xw],
                            feat[g, F:F + 1, t0:t0 + tl, 0:xw],
                        )
                return ftile

            # software pipeline: per slot issue mm1(tau), mm2(tau-L1),
            # mm3(tau-L2) so a stalled mm2/mm3 never head-of-line blocks a
            # ready mm1 in the in-order PE queue. Pipelines across step
            # boundaries too (step t+1's mm1s overlap step t's tail).
            L1, L2 = lag1, lag2
            L2A = lag2a if lag2a is not None else lag2
            L2T = lag2t if lag2t is not None else lag2

            def half_evict_insert(st, glo):
                # evict pd rows (112+4glo..) -> ds, write out, then fire
                # the two groups' delta-insert DMAs for step t+1
                t = st["t"]
                pd = st["pdA"] if glo == 0 else st["pdB"]
                ds = st["dsA"] if glo == 0 else st["dsB"]
                if pd_swap == 'dve':
                    use_act = False
                elif pd_swap == 'alt':
                    use_act = (glo == 0) and (t % 2 == 1)
                else:
                    use_act = (glo % 4 == 0) != pd_swap
                if pd_split:
                    # half-FD on each engine: shortens the critical
                    # evict->insert chain and balances engine load
                    nc.scalar.activation(ds[:, 0:NT // 2],
                                         pd[:, 0:NT // 2], Copy,
                                         bias=float(b3_value))
                    nc.vector.tensor_scalar(ds[:, NT // 2:NT],
                                            pd[:, NT // 2:NT],
                                            float(b3_value), None, add)
                elif use_act:
                    nc.scalar.activation(ds[:, :], pd[:, :], Copy,
                                         bias=float(b3_value))
                else:
                    nc.vector.tensor_scalar(ds[:, :], pd[:, :],
                                            float(b3_value), None, add)
                if st["ftgt"] is not None:
                    # inserts first: they gate step t+1's mm1s; the out-DMA
                    # gates nothing
                    for g in (glo, glo + 1):
                        nc.sync.dma_start(
                            st["ftgt"][32 * g + 8:32 * g + 9,
                                       st["tt1"] * xw:(st["tt1"] + 1) * xw],
                            ds[112 + 4 * g:116 + 4 * g, :],
                        )
                if late_out and glo == 0:
                    st["outA_pending"] = ds
                    return
                if late_out:
                    dsA = st.pop("outA_pending")
                    nc.sync.dma_start(out[t, 0:8, :], dsA[112:120, :])
                nc.sync.dma_start(
                    out[t, 4 * glo:4 * glo + 8, :],
                    ds[112 + 4 * glo:120 + 4 * glo, :])

            pair_state = {}

            def issue_mm2_paired(rec):
                i = rec["i"]
                if not pair_state:
                    pair_state["ph2"] = ph2pool.tile(
                        [128, 2 * NT], f32, tag="ph2",
                        name=f"ph2p_{rec['t']}_{i}")
                    pair_state["h2r"] = h2pool.tile(
                        [128, 2 * NT], bf, tag="h2r",
                        name=f"h2rp_{rec['t']}_{i}")
                    pair_state["n"] = 0
                ph2p = pair_state["ph2"]
                h2rp = pair_state["h2r"]
                half = NT * pair_state["n"]
                nc.tensor.matmul(ph2p[:, half:half + NT], lhsT=w2sb[:],
                                 rhs=rec["h1r"], start=True, stop=True)
                rec["h2r"] = h2rp[:, half:half + NT]
                pair_state["n"] += 1
                if pair_state["n"] == 2:
                    if (i // 2) % 2 == 0:
                        nc.scalar.activation(h2rp[:], ph2p[:], Relu,
                                             bias=b2sb[:, 0:1])
                    else:
                        nc.vector.tensor_scalar(h2rp[:], ph2p[:],
                                                b2sb[:, 0:1], 0.0, add,
                                                amax)
                    pair_state.clear()

            def issue_mm2(rec):
                if pair_ph2:
                    issue_mm2_paired(rec)
                    return
                i = rec["i"]
                ph2 = ph2pool.tile([128, NT], f32, tag="ph2",
                                   name=f"ph2_{rec['t']}_{i}")
                nc.tensor.matmul(ph2[:], lhsT=w2sb[:], rhs=rec["h1r"],
                                 start=True, stop=True)
                h2r = h2pool.tile([128, NT], bf, tag="h2r",
                                  name=f"h2r{rec['t']}_{i}")
                if i in split_h2:
                    # half on each engine: balances totals at finer grain
                    nc.scalar.activation(h2r[:, 0:NT // 2],
                                         ph2[:, 0:NT // 2], Relu,
                                         bias=b2sb[:, 0:1])
                    nc.vector.tensor_scalar(h2r[:, NT // 2:NT],
                                            ph2[:, NT // 2:NT],
                                            b2sb[:, 0:1], 0.0, add, amax)
                elif (i % 2 == 1) != flip or i in act_h2:
                    nc.scalar.activation(h2r[:], ph2[:], Relu,
                                         bias=b2sb[:, 0:1])
                else:
                    nc.vector.tensor_scalar(h2r[:], ph2[:], b2sb[:, 0:1],
                                            0.0, add, amax)
                rec["h2r"] = h2r

            def issue_mm3(rec):
                i = rec["i"]
                st = rec["st"]
                pd = st["pdA"] if i < 8 else st["pdB"]
                nc.tensor.matmul(
                    pd[:],
                    lhsT=w3sb[:, 15 - i:15 - i + 128],
                    rhs=rec["h2r"][:],
                    start=(i % 8 == 0), stop=(i % 8 == 7),
                )
                if i == 7:
                    half_evict_insert(st, 0)
                elif i == 15:
                    half_evict_insert(st, 2)

            for _rep in range(repeats):
                fcur = load_chunk(0)
                fnxt = None
                ring = []
                m3p = 0

                for t in range(t_steps):
                    tt = t % TC
                    if tt == 0 and t > 0:
                        fcur = fnxt
                    if tt == 0 and t + TC < t_steps:
                        fnxt = load_chunk(t // TC + 1)

                    tt1 = (t + 1) % TC
                    st = {
                        "t": t, "tt1": tt1,
                        "ftgt": ((fcur if tt1 > 0 else fnxt)
                                 if t + 1 < t_steps else None),
                        "pdA": pdpool.tile([128, NT], f32, tag="pd",
                                           name=f"pdA{t}"),
                        "pdB": pdpool.tile([128, NT], f32, tag="pd",
                                           name=f"pdB{t}"),
                        "dsA": dpool.tile([128, NT], bf, tag="dst",
                                          name=f"dsA{t}"),
                        "dsB": dpool.tile([128, NT], bf, tag="dst",
                                          name=f"dsB{t}"),
                    }
                    for g in range(4):
                        for s in range(npacks):
                            i = 4 * g + s
                            fs = tt * xw + NT * s
                            ph1 = ph1pool.tile([128, NT], f32, tag="ph1",
                                               name=f"ph1_{t}_{i}")
                            nc.tensor.matmul(
                                ph1[:],
                                lhsT=w1sb[32 * g:32 * g + 9, :],
                                rhs=fcur[32 * g:32 * g + 9, fs:fs + NT],
                                start=True, stop=True,
                                tile_position=(32 * g, 0),
                            )
                            h1r = h1pool.tile([128, NT], bf, tag="h1r",
                                              name=f"h1r{t}_{i}")
                            extra = (i in extra_act and
                                     (not alt_balance or t % 2 == 0))
                            if (i % 2 == 0) != flip or extra:
                                nc.scalar.activation(h1r[:], ph1[:], Relu,
                                                     bias=b1sb[:, 0:1])
                            else:
                                nc.vector.tensor_scalar(h1r[:], ph1[:],
                                                        b1sb[:, 0:1], 0.0,
                                                        add, amax)
                            ring.append({"t": t, "i": i, "st": st,
                                         "h1r": h1r[:]})
                            if len(ring) > L1:
                                issue_mm2(ring[-1 - L1])
                            while m3p < len(ring):
                                rec = ring[m3p]
                                lag = (L2A if rec["i"] < 8 else
                                       (L2T if rec["i"] >= 12 else L2))
                                if len(ring) - 1 - m3p >= lag:
                                    issue_mm3(rec)
                                    ring[m3p] = None
                                    m3p += 1
                                else:
                                    break
                # drain
                for k in range(L1, 0, -1):
                    if len(ring) >= k and ring[-k] is not None:
                        issue_mm2(ring[-k])
                while m3p < len(ring):
                    if ring[m3p] is not None:
                        issue_mm3(ring[m3p])
                        ring[m3p] = None
                    m3p += 1

    nc.compile()
    return nc


def build_kernel_v1(nsh=NSH, t_steps=T_FULL, num_cores=N_CORES, b3_value=0.0,
                    chunk_steps=15, pair_h1=False, pair_pd=False,
                    merge_pd=False, repeats=1):
    """Baseline (V1) per-core Bass graph. Returns the compiled nc."""
    import concourse.bass as bass
    import concourse.tile as tile
    from concourse import bacc, mybir

    bf = mybir.dt.bfloat16
    f32 = mybir.dt.float32
    NT = 512                       # path-tile width (one fp32 psum bank)
    ntiles = nsh // NT
    npacks = ntiles // 4           # pack = 4 row-tiled tiles
    xw = npacks * NT               # per-step free width
    TC = min(chunk_steps, t_steps)
    assert ntiles % 4 == 0

    nc = bacc.Bacc(
        "TRN2", target_bir_lowering=False, debug=False,
        num_devices=num_cores,
    )

    feat = nc.declare_dram_parameter("features", [4, F, t_steps, xw], bf, isOutput=False)
    w1p = nc.declare_dram_parameter("W1p", [128, HID], bf, isOutput=False)
    w1d = nc.declare_dram_parameter("W1d", [128, HID], bf, isOutput=False)
    w2 = nc.declare_dram_parameter("W2", [HID, HID], bf, isOutput=False)
    w3sp = nc.declare_dram_parameter("W3sp", [128, 8, 113], bf, isOutput=False)
    b1 = nc.declare_dram_parameter("b1", [128, 1], f32, isOutput=False)
    b2 = nc.declare_dram_parameter("b2", [128, 1], f32, isOutput=False)
    out = nc.declare_dram_parameter("out", [t_steps, 4, xw], bf, isOutput=True)

    Relu = mybir.ActivationFunctionType.Relu
    Copy = mybir.ActivationFunctionType.Copy
    add = mybir.AluOpType.add
    amax = mybir.AluOpType.max

    with tile.TileContext(nc) as tc:
        with (
            tc.tile_pool(name="consts", bufs=1) as cpool,
            tc.tile_pool(name="f", bufs=2) as fpool,
            tc.tile_pool(name="h1r", bufs=6) as h1pool,
            tc.tile_pool(name="h2r", bufs=4) as h2pool,
            tc.tile_pool(name="dst", bufs=dst_bufs) as dpool,
            tc.tile_pool(name="ph1", bufs=2 if pair_h1 else 4, space="PSUM") as ph1pool,
            tc.tile_pool(name="ph2", bufs=2, space="PSUM") as ph2pool,
            tc.tile_pool(name="pd", bufs=1 if pair_pd else 2, space="PSUM") as pdpool,
        ):
            w1sb = cpool.tile([128, HID], bf, tag="w1")
            w1dsb = cpool.tile([128, HID], bf, tag="w1d")
            w2sb = cpool.tile([HID, HID], bf, tag="w2")
            w3sb = cpool.tile([128, 8, 113], bf, tag="w3")
            b1sb = cpool.tile([128, 1], f32, tag="b1")
            b2sb = cpool.tile([128, 1], f32, tag="b2")
            nc.gpsimd.dma_start(w1sb[:], w1p[:])
            nc.gpsimd.dma_start(w1dsb[:], w1d[:])
            nc.gpsimd.dma_start(w2sb[:], w2[:])
            nc.gpsimd.dma_start(w3sb[:], w3sp[:])
            nc.gpsimd.dma_start(b1sb[:], b1[:])
            nc.gpsimd.dma_start(b2sb[:], b2[:])

            def load_chunk(ci):
                t0 = ci * TC
                tl = min(TC, t_steps - t0)
                ftile = fpool.tile([128, TC * xw], bf, tag="f")
                for g in range(4):
                    nc.sync.dma_start(
                        ftile[32 * g:32 * g + 8, 0:tl * xw],
                        feat[g, 0:F, t0:t0 + tl, 0:xw],
                    )
                return ftile

            for _rep in range(repeats):
              fcur = load_chunk(0)
              fnxt = None
              ds_prev = None

              for t in range(t_steps):
                tt = t % TC
                if tt == 0 and t > 0:
                    fcur = fnxt
                if tt == 0 and t + TC < t_steps:
                    fnxt = load_chunk(t // TC + 1)

                if merge_pd:
                    ds_pairs = [dpool.tile([113, NT], bf, tag="dst",
                                           name=f"ds{t}_{p}")
                                for p in range(npacks // 2)]
                else:
                    ds = dpool.tile([128, xw], bf, tag="dst")
                pd = None
                for s in range(npacks):
                    # ---- mm1: K=8 features (+ K=1 delta), 4x row-tiled;
                    #      pairs (g0,g1) and (g2,g3) share a 2-bank psum ----
                    if pair_h1:
                        pairs = [ph1pool.tile([128, 2 * NT], f32, tag="ph1",
                                              name=f"ph1p{t}_{s}_{p}")
                                 for p in range(2)]
                        ph1ap = [pairs[g // 2][:, NT * (g % 2):NT * (g % 2 + 1)]
                                 for g in range(4)]
                    else:
                        pairs = [ph1pool.tile([128, NT], f32, tag="ph1",
                                              name=f"ph1p{t}_{s}_{p}")
                                 for p in range(4)]
                        ph1ap = [pairs[g][:] for g in range(4)]
                    for g in range(4):
                        fs = tt * xw + NT * s
                        nc.tensor.matmul(
                            ph1ap[g],
                            lhsT=w1sb[32 * g:32 * g + 8, :],
                            rhs=fcur[32 * g:32 * g + 8, fs:fs + NT],
                            start=True, stop=(t == 0),
                            tile_position=(32 * g, 0),
                        )
                    if t > 0:
                        for g in range(4):
                            if merge_pd:
                                r = 32 * g + 16 * (s % 2)
                                lhsT_d = w1dsb[r:r + 1, :]
                                rhs_d = ds_prev[s // 2][r:r + 1, :]
                            else:
                                lhsT_d = w1dsb[32 * g:32 * g + 1, :]
                                rhs_d = ds_prev[32 * g:32 * g + 1, NT * s:NT * (s + 1)]
                            nc.tensor.matmul(
                                ph1ap[g],
                                lhsT=lhsT_d,
                                rhs=rhs_d,
                                start=False, stop=True,
                                tile_position=(32 * g, 0),
                            )
                    # h1 eviction: paired (one op per 2 tiles) or single
                    h1aps = []
                    if pair_h1:
                        for p in range(2):
                            h1r = h1pool.tile([128, 2 * NT], bf, tag="h1r",
                                              name=f"h1r{t}_{s}_{p}")
                            if p == 0:
                                nc.scalar.activation(h1r[:], pairs[p][:], Relu, bias=b1sb[:, 0:1])
                            else:
                                nc.vector.tensor_scalar(h1r[:], pairs[p][:], b1sb[:, 0:1], 0.0, add, amax)
                            h1aps += [h1r[:, 0:NT], h1r[:, NT:2 * NT]]
                    else:
                        for g in range(4):
                            h1r = h1pool.tile([128, NT], bf, tag="h1r",
                                              name=f"h1r{t}_{s}_{g}")
                            if g % 2 == 0:
                                nc.scalar.activation(h1r[:], pairs[g][:], Relu, bias=b1sb[:, 0:1])
                            else:
                                nc.vector.tensor_scalar(h1r[:], pairs[g][:], b1sb[:, 0:1], 0.0, add, amax)
                            h1aps.append(h1r[:])
                    if merge_pd:
                        if s % 2 == 0:
                            pd = pdpool.tile([113, NT], f32, tag="pd",
                                             name=f"pd{t}_{s}")
                        pdh = pd[:]
                    elif pair_pd:
                        if s % 2 == 0:
                            pd = pdpool.tile([97, 2 * NT], f32, tag="pd",
                                             name=f"pd{t}_{s}")
                        pdh = pd[:, NT * (s % 2):NT * (s % 2 + 1)]
                    else:
                        pd = pdpool.tile([97, NT], f32, tag="pd",
                                         name=f"pd{t}_{s}")
                        pdh = pd[:]
                    for g in range(4):
                        ph2 = ph2pool.tile([128, NT], f32, tag="ph2")
                        nc.tensor.matmul(
                            ph2[:], lhsT=w2sb[:],
                            rhs=h1aps[g],
                            start=True, stop=True)
                        h2r = h2pool.tile([128, NT], bf, tag="h2r")
                        if g % 2 == 0:
                            nc.vector.tensor_scalar(h2r[:], ph2[:], b2sb[:, 0:1], 0.0, add, amax)
                        else:
                            nc.scalar.activation(h2r[:], ph2[:], Relu, bias=b2sb[:, 0:1])
                        # delta for tile (s,g) -> pd row 32g + 16*(s%2)
                        if merge_pd:
                            nc.tensor.matmul(
                                pdh,
                                lhsT=w3sb[:, 2 * g + (s % 2), :],
                                rhs=h2r[:],
                                start=(s % 2 == 0 and g == 0),
                                stop=(s % 2 == 1 and g == 3),
                            )
                        else:
                            nc.tensor.matmul(
                                pdh,
                                lhsT=w3sb[:, 2 * g, 0:97],
                                rhs=h2r[:],
                                start=(g == 0), stop=(g == 3),
                            )
                    # delta eviction (+b3)
                    if merge_pd:
                        if s % 2 == 1:
                            p = s // 2
                            if p % 2 == 0:
                                nc.scalar.activation(ds_pairs[p][:], pd[:], Copy,
                                                     bias=float(b3_value))
                            else:
                                nc.vector.tensor_scalar(ds_pairs[p][:], pd[:],
                                                        float(b3_value), None, add)
                            nc.sync.dma_start(
                                out[t, 0:4, 1024 * p:1024 * (p + 1)],
                                ds_pairs[p][0:113:16, :],
                            )
                    elif pair_pd:
                        if s % 2 == 1:
                            dsl = ds[0:97, NT * (s - 1):NT * (s + 1)]
                            if s % 4 == 1:
                                nc.scalar.activation(dsl, pd[:], Copy, bias=float(b3_value))
                            else:
                                nc.vector.tensor_scalar(dsl, pd[:], float(b3_value), None, add)
                    else:
                        dsl = ds[0:97, NT * s:NT * (s + 1)]
                        if s % 2 == 0:
                            nc.scalar.activation(dsl, pd[:], Copy, bias=float(b3_value))
                        else:
                            nc.vector.tensor_scalar(dsl, pd[:], float(b3_value), None, add)
                if merge_pd:
                    ds_prev = ds_pairs
                else:
                    # one out-DMA for the whole step
                    nc.sync.dma_start(out[t], ds[0:97:32, :])
                    ds_prev = ds

    nc.compile()
    return nc


def build_kernel(nsh=NSH, t_steps=T_FULL, num_cores=N_CORES, b3_value=0.0,
                 variant=None, **kw):
    v = variant or VARIANT
    if v == "v1":
        return build_kernel_v1(nsh, t_steps, num_cores, b3_value, **kw)
    return build_kernel_v2(nsh, t_steps, num_cores, b3_value, **kw)


_NC_CACHE = {}


def _get_nc(nsh=NSH, t_steps=T_FULL, num_cores=N_CORES, b3_value=0.0,
            repeats=1):
    key = (VARIANT, nsh, t_steps, num_cores, float(b3_value), repeats)
    if key not in _NC_CACHE:
        _NC_CACHE[key] = build_kernel(nsh, t_steps, num_cores, b3_value,
                                      repeats=repeats)
    return _NC_CACHE[key]


def prep_core_inputs_v1(features, W1, b1, W2, b2, W3, b3, num_cores=N_CORES):
    """Host-side shard + repack for V1. Returns list of per-core in_maps."""
    n, t_steps, f = features.shape
    nsh = n // num_cores
    NT = 512
    npacks = nsh // (4 * NT)
    xw = npacks * NT

    w1p = np.zeros((128, HID), dtype=BF16)
    w1d = np.zeros((128, HID), dtype=BF16)
    for g in range(4):
        w1p[32 * g:32 * g + 8, :] = W1[0:8].astype(BF16)
        w1d[32 * g, :] = W1[8].astype(BF16)
        w1d[32 * g + 16, :] = W1[8].astype(BF16)
    w3sp = np.zeros((128, 8, 113), dtype=BF16)
    for g in range(4):
        for u in range(2):
            w3sp[:, 2 * g + u, 32 * g + 16 * u] = W3[:, 0].astype(BF16)
    w2b = W2.astype(BF16)
    b1c = b1.reshape(128, 1).astype(np.float32)
    b2c = b2.reshape(128, 1).astype(np.float32)

    in_maps = []
    for c in range(num_cores):
        fc = features[c * nsh:(c + 1) * nsh]          # (nsh, T, F)
        # path p = 2048s + 512g + c_ ; fpk[g, k, t, 512s + c_]
        fpk = fc.reshape(npacks, 4, NT, t_steps, f)   # (s, g, c_, t, k)
        fpk = fpk.transpose(1, 4, 3, 0, 2).reshape(4, f, t_steps, xw)
        in_maps.append({
            "features": np.ascontiguousarray(fpk).astype(BF16),
            "W1p": w1p, "W1d": w1d, "W2": w2b, "W3sp": w3sp,
            "b1": b1c, "b2": b2c,
        })
    return in_maps


def prep_core_inputs_v2(features, W1, b1, W2, b2, W3, b3, num_cores=N_CORES):
    """Host-side shard + repack for V2. Returns list of per-core in_maps."""
    n, t_steps, f = features.shape
    nsh = n // num_cores
    NT = 512
    npacks = nsh // (4 * NT)
    xw = npacks * NT

    w1p = np.zeros((128, HID), dtype=BF16)
    for g in range(4):
        w1p[32 * g:32 * g + 8, :] = W1[0:8].astype(BF16)
        w1p[32 * g + 8, :] = W1[8].astype(BF16)
    w3b = np.zeros((128, 144), dtype=BF16)
    w3b[:, 127] = W3[:, 0].astype(BF16)
    w2b = W2.astype(BF16)
    b1c = b1.reshape(128, 1).astype(np.float32)
    b2c = b2.reshape(128, 1).astype(np.float32)

    in_maps = []
    for c in range(num_cores):
        fc = features[c * nsh:(c + 1) * nsh]          # (nsh, T, F)
        fpk = fc.reshape(npacks, 4, NT, t_steps, f)   # (s, g, c_, t, k)
        fpk = fpk.transpose(1, 4, 3, 0, 2).reshape(4, f, t_steps, xw)
        # row F is the delta row: zeros (step 0 reads it; later steps get
        # it overwritten in SBUF by the per-step insert-DMAs)
        fz = np.zeros((4, f + 1, t_steps, xw), dtype=BF16)
        fz[:, 0:f] = fpk.astype(BF16)
        in_maps.append({
            "features": fz,
            "W1p": w1p, "W2": w2b, "W3b": w3b,
            "b1": b1c, "b2": b2c,
        })
    return in_maps


def prep_core_inputs(features, W1, b1, W2, b2, W3, b3, num_cores=N_CORES):
    if VARIANT == "v1":
        return prep_core_inputs_v1(features, W1, b1, W2, b2, W3, b3,
                                   num_cores)
    return prep_core_inputs_v2(features, W1, b1, W2, b2, W3, b3, num_cores)


def gather_out_v1(res_core, nsh, t_steps):
    """(T, 4, xw) bf16 -> (nsh, T) fp32, path p = 2048s + 512g + c."""
    npacks = nsh // 2048
    o = np.asarray(res_core).astype(np.float32)       # (T, 4, xw)
    o = o.reshape(t_steps, 4, npacks, 512)            # (t, g, s, c)
    o = o.transpose(2, 1, 3, 0).reshape(nsh, t_steps)
    return o


def gather_out_v2(res_core, nsh, t_steps):
    """(T, 16, 512) bf16, row j=4g+s -> (nsh, T) fp32, p = 2048s+512g+c."""
    o = np.asarray(res_core).astype(np.float32)       # (T, 16, 512)
    o = o.reshape(t_steps, 4, 4, 512)                 # (t, g, s, c)
    o = o.transpose(2, 1, 3, 0).reshape(nsh, t_steps)
    return o


def gather_out(res_core, nsh, t_steps):
    if VARIANT == "v1":
        return gather_out_v1(res_core, nsh, t_steps)
    return gather_out_v2(res_core, nsh, t_steps)


def run(features, W1, b1, W2, b2, W3, b3, **run_kwargs):
    """Run on the 8 cores; returns (full_output, BassKernelResults)."""
    from concourse.bass_utils import run_bass_kernel_spmd

    features = np.asarray(features)
    n, t_steps, f = features.shape
    nsh = n // N_CORES
    in_maps = prep_core_inputs(features, W1, b1, W2, b2, W3, b3)
    nc = _get_nc(nsh, t_steps, N_CORES, float(np.asarray(b3).reshape(-1)[0]))
    res = run_bass_kernel_spmd(nc, in_maps, core_ids=list(range(N_CORES)), **run_kwargs)
    outs = [gather_out(res.results[c]["out"], nsh, t_steps) for c in range(N_CORES)]
    return np.concatenate(outs, axis=0), res


def kernel(features, W1, b1, W2, b2, W3, b3):
    out, _ = run(features, W1, b1, W2, b2, W3, b3)
    return out
